# revision 7
# baseline (speedup 1.0000x reference)
"""CenterLoss kernel for Trainium2 (8 NeuronCores, Bass).

Reference computation:
    c    = centers[labels]              # [B, D] gather (B=256, D=512)
    dist = sum((x - c)**2, axis=1)      # [B]
    dist = clip(dist, 1e-12, 1e12)
    out  = mean(dist)                   # scalar f32

Sharding strategy (the "all-gather the needed B rows" plan):
  - The gather of the B=256 needed center rows out of the large table is pure
    data movement; it is done host-side while building each core's input shard.
  - Batch is sharded 32 rows/core across 8 cores.
  - Per-core layout: partition p = (row, feature-half) pair (p = 2*row + h),
    free dim carries that half-row's 256 features for x then for c, i.e. one
    [64, 512] bf16 tile = 1 KiB per partition, a single input DMA whose
    descriptors stay >= 512 B (full DMA-engine efficiency).
  - Device: DVE computes d = x - c, sq = d*d, then a free-axis add-reduce
    into [64, 1] fp32 partial sums - all three ops stream back-to-back on one
    engine.  The Scalar (Activation) engine - a HWDGE engine - issues the
    output DMA on the reduce's completion semaphore and holds the program
    until the DMA's completion semaphore lands.  No Tensor engine, no PSUM,
    no drain.
    (Rejected fusions, both tried: tensor_tensor_reduce fails to compile on
    this neuronxcc build - "ISA wrong length" in codegen; and activation
    (Square, accum_out) on the Scalar engine computes the right sums but
    reading the DVE's sub result from another engine right after its
    completion semaphore is a write-visibility race - the semaphore beats
    the data and the Scalar engine squares stale zeros.  DVE->DMA-engine
    reads ~700ns later are safe; engine->engine reads ~100ns later are not.)
  - Host sums the two per-row halves, applies the clip, and takes the mean
    over all 256 rows (the all-reduce step).

Numerics: inputs travel as bf16 (stat error ~1e-3, tolerance 2e-2); the
multiply-accumulate runs in fp32 inside the DVE ALU with an fp32 accumulator.

Hard-won correctness rules baked in here:
  - Some engine MUST wait for the output DMA's completion semaphore before
    falling into the end-of-program barrier, or the runtime reads back the
    output buffer while the write tail is still in flight.  The Scalar engine
    (which issues the DMA) does this wait.
  - Dependent same-engine DVE ops pipeline under relaxed ordering with a fixed
    issue lag; a short consumer overtakes a long producer and reads its tail
    before it is written.  Equal-length streaming pairs (sub -> ttr) are safe;
    the fused reduce is the last DVE op and is only read by the out-DMA
    ~600ns later, which is safe.  No short op may follow it.
"""

import numpy as np
import ml_dtypes

import concourse.bass as bass
import concourse.mybir as mybir
from concourse.bass_utils import run_bass_kernel_spmd

B = 256
D = 512
N_CORES = 8
R = B // N_CORES                      # 32 batch rows per core
HALVES = 2                            # feature halves per row
P = R * HALVES                        # 64 partitions used
FH = D // HALVES                      # 256 features per half

BF16 = ml_dtypes.bfloat16

_nc_cache = None


def _build_nc() -> bass.Bass:
    nc = bass.Bass()
    f32 = mybir.dt.float32
    bf16 = mybir.dt.bfloat16

    big = nc.dram_tensor("big", [P, 2 * FH], bf16, kind="ExternalInput")
    out = nc.dram_tensor("dist", [P, 1], f32, kind="ExternalOutput")

    with (
        nc.sbuf_tensor([P, 2 * FH], bf16) as bs,
        nc.sbuf_tensor([P, FH], bf16) as dt,
        nc.sbuf_tensor([P, FH], bf16) as sq,
        nc.sbuf_tensor([P, 1], f32) as dist_raw,
        nc.semaphore("dsem") as dsem,
        nc.semaphore("vsem") as vsem,
        nc.Block() as block,
    ):
        xt = bs[:, 0:FH]
        ct = bs[:, FH:2 * FH]

        @block.sync
        def _(sync):
            sync.dma_start(out=bs[:], in_=big[:]).then_inc(dsem, 16)

        @block.vector
        def _(vector):
            vector.wait_ge(dsem, 16)
            vector.tensor_sub(dt[:], xt, ct)
            vector.tensor_mul(sq[:], dt[:], dt[:])
            view = sq[:].rearrange("p (a b) -> p a b", a=1, b=FH)
            vector.tensor_reduce(
                dist_raw[:], view, axis=mybir.AxisListType.X,
                op=mybir.AluOpType.add,
            ).then_inc(vsem, 1)

        @block.scalar
        def _(scalar):
            scalar.wait_ge(vsem, 1)
            scalar.dma_start(out=out[:], in_=dist_raw[:]).then_inc(dsem, 16)
            scalar.wait_ge(dsem, 32)

    return nc


def _build_in_maps(x: np.ndarray, labels: np.ndarray, centers: np.ndarray):
    c = centers[labels]                                # [B, D] host-side gather
    xb = x.astype(BF16)
    cb = c.astype(BF16)
    in_maps = []
    for i in range(N_CORES):
        xs = xb[i * R:(i + 1) * R].reshape(P, FH)      # p = 2*row + half
        cs = cb[i * R:(i + 1) * R].reshape(P, FH)
        in_maps.append(
            {"big": np.ascontiguousarray(np.concatenate([xs, cs], axis=1))}
        )
    return in_maps


def kernel(x: np.ndarray, labels: np.ndarray, centers: np.ndarray) -> np.ndarray:
    global _nc_cache
    x = np.asarray(x, dtype=np.float32)
    labels = np.asarray(labels)
    centers = np.asarray(centers, dtype=np.float32)

    in_maps = _build_in_maps(x, labels, centers)

    if _nc_cache is None:
        _nc_cache = _build_nc()

    res = run_bass_kernel_spmd(_nc_cache, in_maps, core_ids=list(range(N_CORES)))

    # [P, 1] partial sums per core -> per-row distances (sum the two halves)
    dist = np.concatenate(
        [
            res.results[i]["dist"].astype(np.float64).reshape(R, HALVES).sum(axis=1)
            for i in range(N_CORES)
        ]
    )
    dist = np.clip(dist, 1e-12, 1e12)
    return np.asarray(dist.mean(), dtype=np.float32)


# revision 10
# speedup vs baseline: 1.0338x; 1.0338x over previous
"""CenterLoss kernel for Trainium2 (8 NeuronCores, Bass).

Reference computation:
    c    = centers[labels]              # [B, D] gather (B=256, D=512)
    dist = sum((x - c)**2, axis=1)      # [B]
    dist = clip(dist, 1e-12, 1e12)
    out  = mean(dist)                   # scalar f32

Sharding strategy (the "all-gather the needed B rows" plan):
  - The gather of the B=256 needed center rows out of the large table is pure
    data movement; it is done host-side while building each core's input shard.
  - Batch is sharded 32 rows/core across 8 cores.
  - Per-core layout: partition p = (row, feature-half) pair (p = 2*row + h),
    free dim carries that half-row's 256 features for x then for c, i.e. one
    [64, 512] bf16 tile = 1 KiB per partition, a single input DMA whose
    descriptors stay >= 512 B (full DMA-engine efficiency).
  - Device: DVE computes d = x - c, sq = d*d, then a free-axis add-reduce
    into [64, 1] fp32 partial sums - all three ops stream back-to-back on one
    engine.  The Sync engine issues the output DMA on the reduce's completion
    semaphore and holds the program until the DMA's completion semaphore
    lands.  No Tensor engine, no PSUM, no drain.
    (The Scalar engine can also dispatch DMAs, but its HWDGE queue
    (qScalarDynamicHW) delivers the 16 per-engine completion-semaphore
    increments ~350ns apart - 5.4us for a 256-byte transfer - while the
    Sync queue delivers them all within ~450ns.  Dispatch DMAs from Sync.)
    (Rejected fusions, both tried: tensor_tensor_reduce fails to compile on
    this neuronxcc build - "ISA wrong length" in codegen; and activation
    (Square, accum_out) on the Scalar engine computes the right sums but
    reading the DVE's sub result from another engine right after its
    completion semaphore is a write-visibility race - the semaphore beats
    the data and the Scalar engine squares stale zeros.  DVE->DMA-engine
    reads ~700ns later are safe; engine->engine reads ~100ns later are not.)
  - Host sums the two per-row halves, applies the clip, and takes the mean
    over all 256 rows (the all-reduce step).

Numerics: inputs travel as bf16 (stat error ~1e-3, tolerance 2e-2); the
multiply-accumulate runs in fp32 inside the DVE ALU with an fp32 accumulator.

Hard-won correctness rules baked in here:
  - Some engine MUST wait for the output DMA's completion semaphore before
    falling into the end-of-program barrier, or the runtime reads back the
    output buffer while the write tail is still in flight.  The Scalar engine
    (which issues the DMA) does this wait.
  - Dependent same-engine DVE ops pipeline under relaxed ordering with a fixed
    issue lag; a short consumer overtakes a long producer and reads its tail
    before it is written.  Equal-length streaming pairs (sub -> ttr) are safe;
    the fused reduce is the last DVE op and is only read by the out-DMA
    ~600ns later, which is safe.  No short op may follow it.
"""

import numpy as np
import ml_dtypes

import concourse.bass as bass
import concourse.mybir as mybir
from concourse.bass_utils import run_bass_kernel_spmd

B = 256
D = 512
N_CORES = 8
R = B // N_CORES                      # 32 batch rows per core
HALVES = 2                            # feature halves per row
P = R * HALVES                        # 64 partitions used
FH = D // HALVES                      # 256 features per half

BF16 = ml_dtypes.bfloat16

_nc_cache = None


def _build_nc() -> bass.Bass:
    nc = bass.Bass()
    f32 = mybir.dt.float32
    bf16 = mybir.dt.bfloat16

    big = nc.dram_tensor("big", [P, 2 * FH], bf16, kind="ExternalInput")
    out = nc.dram_tensor("dist", [P, 1], f32, kind="ExternalOutput")

    with (
        nc.sbuf_tensor([P, 2 * FH], bf16) as bs,
        nc.sbuf_tensor([P, FH], bf16) as dt,
        nc.sbuf_tensor([P, FH], bf16) as sq,
        nc.sbuf_tensor([P, 1], f32) as dist_raw,
        nc.semaphore("dsem") as dsem,
        nc.semaphore("vsem") as vsem,
        nc.Block() as block,
    ):
        xt = bs[:, 0:FH]
        ct = bs[:, FH:2 * FH]

        @block.sync
        def _(sync):
            sync.dma_start(out=bs[:], in_=big[:]).then_inc(dsem, 16)
            sync.wait_ge(vsem, 1)
            sync.dma_start(out=out[:], in_=dist_raw[:]).then_inc(dsem, 16)
            sync.wait_ge(dsem, 32)

        @block.vector
        def _(vector):
            vector.wait_ge(dsem, 16)
            vector.tensor_sub(dt[:], xt, ct)
            vector.tensor_mul(sq[:], dt[:], dt[:])
            view = sq[:].rearrange("p (a b) -> p a b", a=1, b=FH)
            vector.tensor_reduce(
                dist_raw[:], view, axis=mybir.AxisListType.X,
                op=mybir.AluOpType.add,
            ).then_inc(vsem, 1)

    return nc


def _build_in_maps(x: np.ndarray, labels: np.ndarray, centers: np.ndarray):
    c = centers[labels]                                # [B, D] host-side gather
    xb = x.astype(BF16)
    cb = c.astype(BF16)
    in_maps = []
    for i in range(N_CORES):
        xs = xb[i * R:(i + 1) * R].reshape(P, FH)      # p = 2*row + half
        cs = cb[i * R:(i + 1) * R].reshape(P, FH)
        in_maps.append(
            {"big": np.ascontiguousarray(np.concatenate([xs, cs], axis=1))}
        )
    return in_maps


def kernel(x: np.ndarray, labels: np.ndarray, centers: np.ndarray) -> np.ndarray:
    global _nc_cache
    x = np.asarray(x, dtype=np.float32)
    labels = np.asarray(labels)
    centers = np.asarray(centers, dtype=np.float32)

    in_maps = _build_in_maps(x, labels, centers)

    if _nc_cache is None:
        _nc_cache = _build_nc()

    res = run_bass_kernel_spmd(_nc_cache, in_maps, core_ids=list(range(N_CORES)))

    # [P, 1] partial sums per core -> per-row distances (sum the two halves)
    dist = np.concatenate(
        [
            res.results[i]["dist"].astype(np.float64).reshape(R, HALVES).sum(axis=1)
            for i in range(N_CORES)
        ]
    )
    dist = np.clip(dist, 1e-12, 1e12)
    return np.asarray(dist.mean(), dtype=np.float32)


# revision 11
# speedup vs baseline: 1.3061x; 1.2634x over previous
"""CenterLoss kernel for Trainium2 (8 NeuronCores, Bass).

Reference computation:
    c    = centers[labels]              # [B, D] gather (B=256, D=512)
    dist = sum((x - c)**2, axis=1)      # [B]
    dist = clip(dist, 1e-12, 1e12)
    out  = mean(dist)                   # scalar f32

Sharding strategy (the "all-gather the needed B rows" plan):
  - The gather of the B=256 needed center rows out of the large table is pure
    data movement; it is done host-side while building each core's input shard.
  - Batch is sharded 32 rows/core across 8 cores.
  - Per-core layout: partition p = batch row, free dim = 512 x-features then
    512 c-features, i.e. one [32, 1024] bf16 tile = 2 KiB per partition, a
    single input DMA whose 32 descriptors stay >= 512 B each (full DMA-engine
    efficiency, no read-modify-write penalty).
  - Device (all on the DVE): d = x - c; sq = d*d; free-axis add-reduce into
    [32, 1] fp32 per-row distances; a drain barrier; then a 32x32 stream
    transpose that lands all 32 per-row sums in partition 0 as a [1, 32] row.
  - The Sync engine issues the [1, 32] output DMA on the transpose's
    completion semaphore and holds the program until the DMA's completion
    semaphore lands.  No Tensor engine, no PSUM.
  - Host applies the clip and the mean over all 256 rows (the all-reduce).

Numerics: inputs travel as bf16 (per-row error ~1e-3, tolerance 2e-2); the
subtract/multiply run in the DVE ALU and the add-reduce accumulates in fp32.

Hard-won rules baked in here (each violated once and paid for):
  - Some engine MUST wait for the output DMA's completion semaphore before
    falling into the end-of-program barrier, or the runtime reads back the
    output buffer while the write tail is still in flight.
  - Dispatch DMAs ONLY from the Sync engine's queue.  The Scalar engine can
    also dispatch (HWDGE), but its queue delivered the 16 per-engine
    completion-semaphore increments ~350ns apart (5.4us for a 256-byte
    transfer).
  - An SBUF->DRAM DMA whose source spans many partitions serializes at
    ~75ns per partition-descriptor (64-partition source: ~4.9us to the
    completion semaphore; single-partition source: ~450ns).  Get the result
    into ONE partition (stream transpose) before DMAing it out.
  - A cross-engine consumer released by a producer's completion semaphore
    can still read stale SBUF ~100ns later (semaphore beats data): a Scalar
    activation gated on the DVE sub's semaphore squared stale zeros.  Only
    DMA-engine reads ~700ns after the semaphore are safe.  Keep dependent
    compute on ONE engine.
  - Same-engine DVE ops pipeline under relaxed ordering with a fixed issue
    lag: equal-or-slower streaming consumers (sub -> mul -> reduce) are safe,
    but a short op (transpose) after a long producer (reduce) would read
    stale input - separate them with a drain.
  - tensor_tensor_reduce (fused multiply+reduce) does not compile on this
    neuronxcc build ("ISA wrong length" in codegen); use separate ops.
"""

import numpy as np
import ml_dtypes

import concourse.bass as bass
import concourse.mybir as mybir
from concourse.bass_utils import run_bass_kernel_spmd

B = 256
D = 512
N_CORES = 8
R = B // N_CORES                      # 32 batch rows per core = partitions
TS = 32                               # stream-transpose square size

BF16 = ml_dtypes.bfloat16

_nc_cache = None


def _build_nc() -> bass.Bass:
    nc = bass.Bass()
    f32 = mybir.dt.float32
    bf16 = mybir.dt.bfloat16

    big = nc.dram_tensor("big", [R, 2 * D], bf16, kind="ExternalInput")
    out = nc.dram_tensor("dist", [1, R], f32, kind="ExternalOutput")

    with (
        nc.sbuf_tensor([R, 2 * D], bf16) as bs,
        nc.sbuf_tensor([R, D], bf16) as dt,
        nc.sbuf_tensor([R, D], bf16) as sq,
        nc.sbuf_tensor([R, TS], f32) as dist_raw,
        nc.sbuf_tensor([R, TS], f32) as dist_t,
        nc.semaphore("dsem") as dsem,
        nc.semaphore("vsem") as vsem,
        nc.Block() as block,
    ):
        xt = bs[:, 0:D]
        ct = bs[:, D:2 * D]

        @block.sync
        def _(sync):
            sync.dma_start(out=bs[:], in_=big[:]).then_inc(dsem, 16)
            sync.wait_ge(vsem, 1)
            sync.dma_start(out=out[:], in_=dist_t[0:1, :]).then_inc(dsem, 16)
            sync.wait_ge(dsem, 32)

        @block.vector
        def _(vector):
            vector.wait_ge(dsem, 16)
            vector.tensor_sub(dt[:], xt, ct)
            vector.tensor_mul(sq[:], dt[:], dt[:])
            view = sq[:].rearrange("p (a b) -> p a b", a=1, b=D)
            vector.tensor_reduce(
                dist_raw[:, 0:1], view, axis=mybir.AxisListType.X,
                op=mybir.AluOpType.add,
            )
            vector.drain()
            vector.transpose(dist_t[:], dist_raw[:]).then_inc(vsem, 1)

    return nc


def _build_in_maps(x: np.ndarray, labels: np.ndarray, centers: np.ndarray):
    c = centers[labels]                                # [B, D] host-side gather
    xb = x.astype(BF16)
    cb = c.astype(BF16)
    in_maps = []
    for i in range(N_CORES):
        xs = xb[i * R:(i + 1) * R]                     # [32, 512]
        cs = cb[i * R:(i + 1) * R]
        in_maps.append(
            {"big": np.ascontiguousarray(np.concatenate([xs, cs], axis=1))}
        )
    return in_maps


def kernel(x: np.ndarray, labels: np.ndarray, centers: np.ndarray) -> np.ndarray:
    global _nc_cache
    x = np.asarray(x, dtype=np.float32)
    labels = np.asarray(labels)
    centers = np.asarray(centers, dtype=np.float32)

    in_maps = _build_in_maps(x, labels, centers)

    if _nc_cache is None:
        _nc_cache = _build_nc()

    res = run_bass_kernel_spmd(_nc_cache, in_maps, core_ids=list(range(N_CORES)))

    dist = np.concatenate(
        [res.results[i]["dist"][0].astype(np.float64) for i in range(N_CORES)]
    )
    dist = np.clip(dist, 1e-12, 1e12)
    return np.asarray(dist.mean(), dtype=np.float32)
